# revision 35
# baseline (speedup 1.0000x reference)
"""GNN GraphConv x2 + Linear on 8 TRN2 cores.

Strategy (graph/data parallel):
- Cores own contiguous node-id blocks of N/8 nodes.  Because the layer-1
  gather windows (src id // span1) and the layer-2 gather windows
  (gslot // span2, with span2 = two cores' slot range) then cut at the same
  core-pair boundaries, every edge has ONE window k for both layers.
- Within a core, nodes are bin-packed into buckets of <=32 nodes with a
  strict cap of <=128 edges per (bucket, window) -- so each (bucket, window)
  is exactly one 128-lane gather chunk and there are no overflow chunks.
- Device, per layer: dma_gather edge source rows (fp32, 256B tokens) into
  [128, C, 64] chunks, cast to bf16 on DVE/ScalarE; one-hot dst-slot
  matrices built with iota+is_equal on DVE (bf16); aggregation
  agg_T[f, slot] += Xg.T @ S on TensorE (bf16, 1 cycle/row) into PSUM
  groups of 16 buckets (512 slots = one PSUM bank); W_rel @ agg_T +
  W_root @ x_T with bf16 weights at N=512, bias+relu on ScalarE, all
  feature-major.  Between layers h1 rows are transposed back (TensorE)
  and exchanged with an AllGather so any core can gather any row.
- Output is feature-major [3, SLOTS] per core in fp32; host inverse-permutes.
"""

import numpy as np

import concourse.bacc as bacc
import concourse.bass as bass
import concourse.tile as tile
from concourse import mybir
from concourse.masks import make_identity

P = 128           # partitions / edge-chunk size
D = 64            # feature dim
BN = 32           # node slots per bucket
K = 4             # gather windows
GB = 16           # buckets per PSUM group
SUPER = 2         # groups per gather super-group
GSLOT = GB * BN   # 512 node slots per group (one PSUM bank)
SGR = SUPER * GB  # buckets (= chunk cols per window) per super-group = 32
PAD_DSTL = 99999.0  # dst_local for padding edge slots (matches no iota slot)

F32 = mybir.dt.float32
BF16 = mybir.dt.bfloat16
I16 = mybir.dt.int16


class Cfg:
    def __init__(self, n_nodes, n_cores, nb_per_core):
        self.n_nodes = n_nodes
        self.n_cores = n_cores
        self.nb = nb_per_core                 # buckets per core
        assert self.nb % SGR == 0
        self.slots = self.nb * BN             # node slots per core
        self.groups = self.nb // GB
        self.supers = self.nb // SGR
        self.ch = K * self.nb                 # chunk cols per core
        self.gslots = self.n_cores * self.slots

    def windows(self, n_src):
        span = -(-n_src // K)
        assert span <= 32768, (n_src, span)
        return span, [(k * span, min(span, n_src - k * span)) for k in range(K)]


# ---------------------------------------------------------------- host side

def _pack_core(node_ids, wcnt, nb):
    """Bin-pack one core's nodes into <=nb buckets: <=BN nodes and <=P edges
    per window.  Returns (bucket_local, slot_in_bucket) or None."""
    import heapq
    order = node_ids[np.argsort(-wcnt[node_ids].max(axis=1), kind="stable")]
    bl = np.empty(len(node_ids), np.int64)
    sib = np.empty(len(node_ids), np.int64)
    base = node_ids[0]
    nodes_in = np.zeros(nb, np.int64)
    win_in = np.zeros((nb, K), np.int64)
    heap = [(0, b) for b in range(nb)]
    heapq.heapify(heap)
    for v in order:
        wc = wcnt[v]
        popped = []
        placed = False
        while heap:
            e, b = heapq.heappop(heap)
            if nodes_in[b] < BN and np.all(win_in[b] + wc <= P):
                bl[v - base] = b
                sib[v - base] = nodes_in[b]
                nodes_in[b] += 1
                win_in[b] += wc
                if nodes_in[b] < BN:
                    heapq.heappush(heap, (int(win_in[b].sum()), b))
                placed = True
                break
            popped.append((e, b))
            if len(popped) > 64:
                break
        for item in popped:
            heapq.heappush(heap, item)
        if not placed:
            return None
    return bl, sib


def prepare(x, edge_index, W1_rel, b1_rel, W1_root, W2_rel, b2_rel, W2_root,
            W_lin, b_lin, n_cores=8):
    """Host preprocessing: returns (cfg, in_maps, meta)."""
    n_nodes = x.shape[0]
    src = np.asarray(edge_index[0], np.int64)
    dst = np.asarray(edge_index[1], np.int64)
    n_edges = src.shape[0]
    assert n_nodes % n_cores == 0
    npc = n_nodes // n_cores
    span1 = -(-n_nodes // K)
    assert span1 * (K - 1) < n_nodes
    # core boundaries must align with window boundaries (2 cores per window)
    assert (2 * npc) % span1 == 0 or span1 == 2 * npc

    win = np.minimum(src // span1, K - 1)
    wcnt = np.bincount(dst * K + win, minlength=n_nodes * K) \
        .reshape(n_nodes, K).astype(np.int64)

    nb = SGR * ((448 + SGR - 1) // SGR)
    while True:
        packs = []
        for c in range(n_cores):
            ids = np.arange(c * npc, (c + 1) * npc)
            res = _pack_core(ids, wcnt, nb)
            if res is None:
                break
            packs.append(res)
        if len(packs) == n_cores:
            break
        nb += SGR
        assert nb * BN * 2 <= 32768, "layer-2 window exceeds int16 range"

    cfg = Cfg(n_nodes, n_cores, nb)
    bucket_local = np.concatenate([p[0] for p in packs])
    slot_in_bucket = np.concatenate([p[1] for p in packs])
    core_of_node = np.arange(n_nodes) // npc
    slot_of_node = bucket_local * BN + slot_in_bucket
    gslot_of_node = core_of_node * cfg.slots + slot_of_node

    span2, _ = cfg.windows(cfg.gslots)
    assert np.all(np.minimum(gslot_of_node[src] // span2, K - 1) == win)

    # per-edge chunk (col, lane) assignment
    bg = core_of_node[dst] * nb + bucket_local[dst]          # global bucket
    ekey = bg * K + win
    eorder = np.argsort(ekey, kind="stable")
    ks = ekey[eorder]
    starts = np.searchsorted(ks, np.arange(n_cores * nb * K + 1))
    lane_sorted = np.arange(n_edges) - starts[ks]
    assert lane_sorted.max() < P
    lane = np.empty(n_edges, np.int64)
    lane[eorder] = lane_sorted

    ul = bucket_local[dst]
    col = (ul // SGR) * (K * SGR) + win * SGR + (ul % SGR)
    ecore = core_of_node[dst]

    gidx = np.zeros((2, n_cores, P, cfg.ch), np.int16)
    dstl = np.full((n_cores, P, cfg.ch), PAD_DSTL, np.float32)
    gidx[0, ecore, lane, col] = (src - win * span1).astype(np.int16)
    gidx[1, ecore, lane, col] = (gslot_of_node[src] - win * span2).astype(np.int16)
    dstl[ecore, lane, col] = slot_in_bucket[dst].astype(np.float32)

    # wrap gather indices into the [16, num/16] call layout (replicated
    # across the 8 gpsimd cores' partition groups).
    cw = SGR * P // 16
    gwrap = np.zeros((2, n_cores, P, cfg.supers * K * cw), np.int16)
    for li in range(2):
        for sg in range(cfg.supers):
            for k in range(K):
                c0 = sg * K * SGR + k * SGR
                cols = gidx[li, :, :, c0:c0 + SGR]            # [C, P, SGR]
                vals = cols.transpose(0, 2, 1).reshape(n_cores, -1)
                blk = vals.reshape(n_cores, -1, 16).transpose(0, 2, 1)
                ci = (sg * K + k) * cw
                for rep in range(P // 16):
                    gwrap[li, :, rep * 16:(rep + 1) * 16, ci:ci + cw] = blk

    # x rows per slot, transposed, per core (bf16 for the root matmul)
    import ml_dtypes
    xpermT = np.zeros((n_cores, D, cfg.slots), ml_dtypes.bfloat16)
    xpermT[core_of_node, :, slot_of_node] = \
        np.asarray(x, np.float32).astype(ml_dtypes.bfloat16)

    import ml_dtypes
    bf = lambda a: np.ascontiguousarray(
        np.asarray(a, np.float32).T.astype(ml_dtypes.bfloat16))
    xpad = np.zeros((n_nodes, 2 * D), dtype=ml_dtypes.bfloat16)
    xpad[:, :D] = np.asarray(x, np.float32).astype(ml_dtypes.bfloat16)
    common = {
        "xfull": xpad.view(np.float32),
        "w1relT": bf(W1_rel),
        "w1rootT": bf(W1_root),
        "w2relT": bf(W2_rel),
        "w2rootT": bf(W2_root),
        "wlinT": bf(W_lin),
        "b1": np.asarray(b1_rel, np.float32).reshape(D, 1).copy(),
        "b2": np.asarray(b2_rel, np.float32).reshape(D, 1).copy(),
        "blin": np.asarray(b_lin, np.float32).reshape(3, 1).copy(),
    }
    in_maps = []
    for c in range(n_cores):
        m = dict(common)
        m["gidx1"] = np.ascontiguousarray(gwrap[0, c])
        m["gidx2"] = np.ascontiguousarray(gwrap[1, c])
        m["dstl"] = np.ascontiguousarray(dstl[c])
        m["xpermT"] = np.ascontiguousarray(xpermT[c])
        in_maps.append(m)

    meta = (core_of_node, slot_of_node)
    return cfg, in_maps, meta


def unshard(results, cfg, meta):
    core_of_node, slot_of_node = meta
    outT = np.stack([results[c]["outT"] for c in range(cfg.n_cores)])
    return np.ascontiguousarray(outT[core_of_node, :, slot_of_node])


# -------------------------------------------------------------- device side

def build_program(cfg, only_gather=False, skip_collective=False,
                  skip_gather=False, repeat=1):
    nc = bacc.Bacc("TRN2", target_bir_lowering=False, debug=False,
                   num_devices=cfg.n_cores)
    f = F32
    SGCH = K * SGR          # chunk cols per super-group = 128
    NIDX = SGR * P          # idxs per gather call = 4096
    CW = NIDX // 16         # idx cols per call = 256
    NW = cfg.supers * K * CW
    xfull = nc.dram_tensor("xfull", [cfg.n_nodes, D], f, kind="ExternalInput")
    gidx1 = nc.dram_tensor("gidx1", [P, NW], I16, kind="ExternalInput")
    gidx2 = nc.dram_tensor("gidx2", [P, NW], I16, kind="ExternalInput")
    dstl = nc.dram_tensor("dstl", [P, cfg.ch], f, kind="ExternalInput")
    xpermT = nc.dram_tensor("xpermT", [D, cfg.slots], BF16, kind="ExternalInput")
    w1relT = nc.dram_tensor("w1relT", [D, D], BF16, kind="ExternalInput")
    w1rootT = nc.dram_tensor("w1rootT", [D, D], BF16, kind="ExternalInput")
    w2relT = nc.dram_tensor("w2relT", [D, D], BF16, kind="ExternalInput")
    w2rootT = nc.dram_tensor("w2rootT", [D, D], BF16, kind="ExternalInput")
    wlinT = nc.dram_tensor("wlinT", [D, 3], BF16, kind="ExternalInput")
    b1 = nc.dram_tensor("b1", [D, 1], f, kind="ExternalInput")
    b2 = nc.dram_tensor("b2", [D, 1], f, kind="ExternalInput")
    blin = nc.dram_tensor("blin", [3, 1], f, kind="ExternalInput")
    outT = nc.dram_tensor("outT", [3, cfg.slots], f, kind="ExternalOutput")

    h1own = nc.dram_tensor("h1own", [cfg.slots, D], f)
    h1ownT = nc.dram_tensor("h1ownT", [D, cfg.slots], BF16)
    h1all = nc.dram_tensor("h1all", [cfg.gslots, D], f, addr_space="Shared")

    Relu = mybir.ActivationFunctionType.Relu
    Copy = mybir.ActivationFunctionType.Copy
    Identity = mybir.ActivationFunctionType.Identity
    _, wins1 = cfg.windows(cfg.n_nodes)
    _, wins2 = cfg.windows(cfg.gslots)

    with tile.TileContext(nc) as tc:
        with (
            tc.tile_pool(name="static", bufs=1) as st_pool,
            tc.tile_pool(name="gst", bufs=2) as gst_pool,
            tc.tile_pool(name="xg", bufs=2) as xg_pool,
            tc.tile_pool(name="selr", bufs=2) as selr_pool,
            tc.tile_pool(name="drain", bufs=2) as dr_pool,
            tc.tile_pool(name="root", bufs=2) as root_pool,
            tc.tile_pool(name="outs", bufs=2) as out_pool,
            tc.tile_pool(name="pagg", bufs=2, space="PSUM") as pagg_pool,
            tc.tile_pool(name="ph", bufs=2, space="PSUM") as ph_pool,
            tc.tile_pool(name="pmisc", bufs=2, space="PSUM") as pmisc_pool,
        ):
            def load(name, dram, shape, dtype=BF16):
                t = st_pool.tile(shape, dtype, name=name)
                nc.sync.dma_start(out=t[:], in_=dram[:])
                return t

            sb_w1relT = load("sb_w1relT", w1relT, [D, D])
            sb_w1rootT = load("sb_w1rootT", w1rootT, [D, D])
            sb_w2relT = load("sb_w2relT", w2relT, [D, D])
            sb_w2rootT = load("sb_w2rootT", w2rootT, [D, D])
            sb_wlinT = load("sb_wlinT", wlinT, [D, 3])
            sb_b1 = load("sb_b1", b1, [D, 1], dtype=f)
            sb_b2 = load("sb_b2", b2, [D, 1], dtype=f)
            sb_blin = load("sb_blin", blin, [3, 1], dtype=f)

            sb_iota = st_pool.tile([P, BN], f, name="sb_iota")
            nc.gpsimd.iota(sb_iota[:], pattern=[[1, BN]], base=0,
                           channel_multiplier=0,
                           allow_small_or_imprecise_dtypes=True)
            sb_ident = st_pool.tile([P, P], f, name="sb_ident")
            make_identity(nc, sb_ident[:])

            import itertools
            for rep, layer in itertools.product(range(repeat), range(2)):
                src_t = xfull if layer == 0 else h1all
                gidx_t = gidx1 if layer == 0 else gidx2
                wrel = sb_w1relT if layer == 0 else sb_w2relT
                wroot = sb_w1rootT if layer == 0 else sb_w2rootT
                bias = sb_b1 if layer == 0 else sb_b2
                wins = wins1 if layer == 0 else wins2

                for sg in range(cfg.supers):
                    gi_sb = gst_pool.tile([P, K * CW], I16, name="gi_sb")
                    nc.sync.dma_start(
                        out=gi_sb[:],
                        in_=gidx_t[:, sg * K * CW:(sg + 1) * K * CW])
                    dl_sb = gst_pool.tile([P, SGCH], f, name="dl_sb")
                    nc.sync.dma_start(
                        out=dl_sb[:],
                        in_=dstl[:, sg * SGCH:(sg + 1) * SGCH])
                    xg = xg_pool.tile([P, SGCH, D], f, name="xg")
                    selr = selr_pool.tile([P, K, SGR, BN], BF16, name="selr")
                    for k in range(K):
                        base, win = wins[k]
                        if skip_gather:
                            nc.vector.memset(xg[:, k * SGR:(k + 1) * SGR, :],
                                             0.0)
                        else:
                            nc.gpsimd.dma_gather(
                                out_ap=xg[:, k * SGR:(k + 1) * SGR, :],
                                in_ap=src_t[base:base + win, :],
                                idxs_ap=gi_sb[:, k * CW:(k + 1) * CW],
                                num_idxs=NIDX,
                                num_idxs_reg=NIDX,
                                elem_size=D,
                                single_packet=False,
                            )
                        nc.vector.tensor_tensor(
                            out=selr[:, k],
                            in0=sb_iota[:, :BN].unsqueeze(1)
                                .broadcast_to([P, SGR, BN]),
                            in1=dl_sb[:, k * SGR:(k + 1) * SGR]
                                .unsqueeze(-1).broadcast_to([P, SGR, BN]),
                            op=mybir.AluOpType.is_equal,
                        )
                    for gl in range(SUPER):
                        if only_gather:
                            continue
                        g = sg * SUPER + gl
                        pagg = pagg_pool.tile([D, GSLOT], f, name="pagg")
                        for b in range(GB):
                            j = gl * GB + b
                            for k in range(K):
                                nc.tensor.matmul(
                                    out=pagg[:, b * BN:(b + 1) * BN],
                                    lhsT=xg[:, k * SGR + j, :]
                                        .bitcast(BF16)[:, :D],
                                    rhs=selr[:, k, j, :],
                                    start=(k == 0), stop=(k == K - 1),
                                    skip_group_check=True,
                                )
                        aggT = dr_pool.tile([D, GSLOT], BF16, name="aggT")
                        nc.scalar.activation(out=aggT[:], in_=pagg[:],
                                             func=Copy)
                        root_rhs = root_pool.tile([D, GSLOT], BF16,
                                                  name="rootst")
                        rsrc = xpermT if layer == 0 else h1ownT
                        nc.sync.dma_start(
                            out=root_rhs[:],
                            in_=rsrc[:, g * GSLOT:(g + 1) * GSLOT])
                        ph = ph_pool.tile([D, GSLOT], f, name="ph")
                        nc.tensor.matmul(out=ph[:], lhsT=wrel[:], rhs=aggT[:],
                                         start=True, stop=False)
                        nc.tensor.matmul(out=ph[:], lhsT=wroot[:],
                                         rhs=root_rhs[:], start=False,
                                         stop=True)
                        if layer == 0:
                            hsl = dr_pool.tile([D, GSLOT], f, name="hsl")
                            nc.scalar.activation(out=hsl[:], in_=ph[:],
                                                 func=Relu, bias=bias[:, :1])
                            hslb = dr_pool.tile([D, GSLOT], BF16, name="hslb")
                            nc.vector.tensor_copy(out=hslb[:], in_=hsl[:])
                            nc.sync.dma_start(
                                out=h1ownT[:, g * GSLOT:(g + 1) * GSLOT],
                                in_=hslb[:])
                            hr = dr_pool.tile([P, GSLOT // P, 2 * D], BF16,
                                              name="hr")
                            nc.vector.memset(hr[:, :, D:], 0.0)
                            for q in range(GSLOT // P):
                                ptr = pmisc_pool.tile([P, D], f, name="ptr",
                                                      tag="pmisc")
                                nc.tensor.transpose(
                                    out=ptr[:],
                                    in_=hsl[:, q * P:(q + 1) * P],
                                    identity=sb_ident[:D, :D])
                                nc.scalar.activation(out=hr[:, q, :D],
                                                     in_=ptr[:], func=Copy)
                            nc.sync.dma_start(
                                out=h1own[g * GSLOT:(g + 1) * GSLOT, :]
                                    .rearrange("(q p) d -> p q d", p=P),
                                in_=hr[:].bitcast(f))
                        else:
                            h2T = dr_pool.tile([D, GSLOT], BF16, name="h2T")
                            nc.scalar.activation(out=h2T[:], in_=ph[:],
                                                 func=Relu, bias=bias[:, :1])
                            po = pmisc_pool.tile([3, GSLOT], f, name="po",
                                                 tag="pmisc")
                            nc.tensor.matmul(out=po[:], lhsT=sb_wlinT[:],
                                             rhs=h2T[:], start=True, stop=True)
                            ot = out_pool.tile([3, GSLOT], f, name="ot")
                            nc.scalar.activation(out=ot[:], in_=po[:],
                                                 func=Identity,
                                                 bias=sb_blin[:, :1])
                            nc.sync.dma_start(
                                out=outT[:, g * GSLOT:(g + 1) * GSLOT],
                                in_=ot[:])

                if layer == 0 and not (skip_collective or only_gather):
                    nc.gpsimd.collective_compute(
                        "AllGather", mybir.AluOpType.bypass,
                        replica_groups=[list(range(cfg.n_cores))],
                        ins=[h1own[:]], outs=[h1all[:]])

            if only_gather:
                zt = st_pool.tile([P, GSLOT], f, name="zt")
                nc.vector.memset(zt[:], 0.0)
                nc.sync.dma_start(out=outT[:, :GSLOT], in_=zt[:3, :])

    nc.compile()
    return nc


# ------------------------------------------------------------------ harness

def kernel(**inputs):
    """Full-input entry point: shards across 8 TRN2 cores, runs the Bass
    kernel via run_bass_kernel_spmd, returns the full [N, 3] float32 output."""
    from concourse.bass_utils import run_bass_kernel_spmd

    np_in = {k: np.asarray(v) for k, v in inputs.items()}
    cfg, in_maps, meta = prepare(
        np_in["x"], np_in["edge_index"],
        np_in["W1_rel"], np_in["b1_rel"], np_in["W1_root"],
        np_in["W2_rel"], np_in["b2_rel"], np_in["W2_root"],
        np_in["W_lin"], np_in["b_lin"], n_cores=8)
    nc = build_program(cfg)
    r = run_bass_kernel_spmd(nc, in_maps, core_ids=list(range(8)))
    return unshard(r.results, cfg, meta)
